# revision 15
# baseline (speedup 1.0000x reference)
"""Sparse attention (sparsemax) TRN2 kernel — 8 NeuronCores, SPMD.

Panel-pipelined v3. Core c handles batch b=c//4 and head pair
(2*(c%4), 2*(c%4)+1). Host sums 4 partials per batch + bias.

Key PE trick: matmul streaming costs ~N cycles, but a CHANGED stationary
operand serializes ~LDWEIGHTS in front (measured 485ns vs 216ns for
N=512 f16). So everything is structured to reuse the stationary operand:
  - phase C sim uses lhsT = qT2[:, t] (both heads stacked, K=128) reused
    across 8 matmuls; the per-head split moves to the rhs via
    zero-padded kZ0/kZ1 (head h's K rows live at partitions h*64..h*64+63,
    the other 64 rows are zero, so the full-K contraction yields that
    head's sim exactly).
  - projections run kc-major with 4 concurrent PSUM accumulators so each
    weight chunk is loaded once and streamed over 4 x-column blocks.
simT keeps the augmented-row trick (row 64: ones/-tau -> PSUM holds
sim - tau) but zero-pads K to 128 to stay in the fast weight-load path.

Pipeline: queries in 4 panels of 512. C(p) sim+MAX8 (top-8 per
1024-half direct from PSUM), DVE chain -> tau, then P2(p): simT,
relu-copies to attnT (split scalar/DVE), attn@v accumulation, fused
output projection. C(p+1) quads interleave into P2(p)'s PE stream.
"""
import sys

sys.path.insert(0, "/opt/trn_rl_repo")

import numpy as np
import concourse.bass as bass
import concourse.bacc as bacc
import concourse.mybir as mybir
import concourse.tile as tile
from concourse.bass_utils import run_bass_kernel_spmd

F32 = mybir.dt.float32
F16 = mybir.dt.float16
A = mybir.AluOpType
AF = mybir.ActivationFunctionType
AX = mybir.AxisListType

B, N, D = 2, 2048, 512
H, DH = 8, 64
SCALE = DH ** -0.5
NT = N // 128        # 16 query tiles
NKB = N // 128       # 16 key blocks
KC = D // 128        # 4 contraction chunks over model dim
NC = 16              # candidates per row
NP = 4               # query panels
PT = NT // NP        # 4 query tiles per panel


def build():
    nc = bacc.Bacc(None, target_bir_lowering=False)

    xT_ext = nc.declare_dram_parameter("xT", [D, N], F16, isOutput=False)
    wq_ext = nc.declare_dram_parameter("wq", [128, KC * 128], F16, isOutput=False)
    wk_ext = nc.declare_dram_parameter("wk", [128, KC * 128], F16, isOutput=False)
    wv_ext = nc.declare_dram_parameter("wv", [128, KC * 128], F16, isOutput=False)
    wo_ext = nc.declare_dram_parameter("wo", [128, D], F16, isOutput=False)
    idn_ext = nc.declare_dram_parameter("idn", [128, 128], F16, isOutput=False)
    rcj_ext = nc.declare_dram_parameter("rcj", [128, NC], F32, isOutput=False)
    out_ext = nc.declare_dram_parameter("out", [N, D], F16, isOutput=True)

    with tile.TileContext(nc) as tc:
        with (
            tc.tile_pool(name="persist", bufs=1) as pp,
            tc.tile_pool(name="statpool", bufs=1) as stp,
            tc.tile_pool(name="attnT", bufs=36) as atp,
            tc.tile_pool(name="outp", bufs=2) as op_,
        ):
            # ---------------- Phase A: loads ----------------
            qT2 = pp.tile([128, N], F16, tag="qT2")
            # per-head zero-padded K tiles for phase C (shared-lhsT trick)
            kZh = [pp.tile([128, N], F16, tag=f"kZ{hh}", name=f"kZ{hh}")
                   for hh in range(2)]
            # K=128-padded augmented tiles for simT
            kAh = [pp.tile([128, N], F16, tag=f"kA{hh}", name=f"kA{hh}")
                   for hh in range(2)]
            qAh = [pp.tile([128, N], F16, tag=f"qA{hh}", name=f"qA{hh}")
                   for hh in range(2)]
            v16 = pp.tile([128, NKB, 128], F16, tag="v16")
            aoT = pp.tile([128, N], F16, tag="aoT")
            wo16 = pp.tile([128, 512], F16, tag="wo")
            idn16 = pp.tile([128, 128], F16, tag="idn")

            stage_ctx = tc.tile_pool(name="stage", bufs=1)
            stg = stage_ctx.__enter__()
            w16 = {nm: stg.tile([128, KC, 128], F16, tag=nm, name=nm)
                   for nm in ("wk", "wq", "wv")}
            xT16 = [stg.tile([128, N], F16, tag=f"xT{kc}", name=f"xT{kc}")
                    for kc in range(KC)]
            # weights are host-repacked so each w16 tile is one contiguous
            # [128, KC*128] region; split by partition halves across the two
            # HWDGE queues. wk + xT0 gate the first projection group.
            nc.sync.dma_start(w16["wk"][0:64, :, :], wk_ext[0:64, :])
            nc.scalar.dma_start(w16["wk"][64:128, :, :], wk_ext[64:128, :])
            for kc in range(KC):
                x0 = kc * 128
                nc.scalar.dma_start(xT16[kc][0:64, :],
                                    xT_ext[x0:x0 + 64, :])
                nc.sync.dma_start(xT16[kc][64:128, :],
                                  xT_ext[x0 + 64:x0 + 128, :])
                if kc == 0:
                    nc.sync.dma_start(w16["wq"][0:64, :, :], wq_ext[0:64, :])
                    nc.scalar.dma_start(w16["wq"][64:128, :, :],
                                        wq_ext[64:128, :])
            nc.gpsimd.dma_start(w16["wv"][:, :, :], wv_ext[:, :])
            vT = stg.tile([128, N], F16, tag="vT", name="vT")
            warm = stg.tile([128, 128], F16, tag="warm", name="warm")
            nc.gpsimd.memset(warm[:], 0.25)
            nc.sync.dma_start(wo16[:], wo_ext[:])
            nc.sync.dma_start(idn16[:], idn_ext[:])
            for hh in range(2):
                nc.gpsimd.memset(kZh[hh][(1 - hh) * 64:(2 - hh) * 64, :], 0.0)
                nc.gpsimd.memset(kAh[hh][64:128, :], 0.0)
                nc.gpsimd.memset(kAh[hh][64:65, :], 1.0)
                nc.gpsimd.memset(qAh[hh][64:128, :], 0.0)

            # ---------------- Phase B: projections ----------------
            # kc-major with 4 live accumulators: each weight chunk is the
            # stationary operand for 4 consecutive matmuls (one LDW amortized).
            # psS opens first so the PSUM stack can reuse psB's banks for
            # psT/psA after the wv group (emitted inside the C(0) lead-in).
            psS_ctx = tc.tile_pool(name="psS", bufs=2, space=bass.MemorySpace.PSUM)
            psS = psS_ctx.__enter__()
            psB_ctx = tc.tile_pool(name="psB", bufs=4, space=bass.MemorySpace.PSUM)
            psB = psB_ctx.__enter__()
            if True:
                # HAM warm-up: keep the PE busy >3.4us during the load wait so
                # the clock gate opens before the projection matmuls arrive
                wps = psB.tile([128, 128], F32, tag="pskq", name="wps")
                for wi in range(36):
                    nc.tensor.matmul(wps[:], warm[:], warm[:],
                                     start=(wi == 0), stop=(wi == 35),
                                     skip_group_check=True)
                wsink = stg.tile([128, 8], F16, tag="wsink", name="wsink")
                nc.scalar.activation(wsink[:], wps[:, 0:8], AF.Copy)
                for nm in ("wk", "wq"):
                    ps4 = [psB.tile([128, 512], F32, tag="pskq", name=f"ps{nb}")
                           for nb in range(4)]
                    for kc in range(KC):
                        for nb in range(4):
                            nc.tensor.matmul(
                                ps4[nb][:], w16[nm][:, kc, :],
                                xT16[kc][:, nb * 512:(nb + 1) * 512],
                                start=(kc == 0), stop=(kc == KC - 1),
                                skip_group_check=True,
                            )
                    for nb in range(4):
                        cols = slice(nb * 512, (nb + 1) * 512)
                        if nm == "wk":
                            # split to the two zero-padded per-head tiles
                            nc.scalar.activation(
                                kZh[0][0:64, cols], ps4[nb][0:64, :], AF.Copy)
                            nc.scalar.activation(
                                kZh[1][64:128, cols], ps4[nb][64:128, :], AF.Copy)
                        else:
                            nc.scalar.activation(qT2[:, cols], ps4[nb][:], AF.Copy)
                for hh in range(2):
                    nc.sync.dma_start(kAh[hh][0:64, :],
                                      kZh[hh][hh * 64:(hh + 1) * 64, :])
                    nc.gpsimd.dma_start(qAh[hh][0:64, :],
                                        qT2[hh * 64:(hh + 1) * 64, :])

                def emit_wv_nb(nb):
                    ps = psB.tile([128, 512], F32, tag="pskq", name="psv")
                    for kc in range(KC):
                        nc.tensor.matmul(
                            ps[:], w16["wv"][:, kc, :],
                            xT16[kc][:, nb * 512:(nb + 1) * 512],
                            start=(kc == 0), stop=(kc == KC - 1),
                        )
                    nc.scalar.activation(
                        vT[:, nb * 512:(nb + 1) * 512], ps[:], AF.Copy)

            # ---------------- tau state ----------------
            st_all, cds_all, rC_all, sC_all, c32_all = {}, {}, {}, {}, {}
            rcj32 = stp.tile([128, NC], F32, tag="rcj", name="rcj")
            nc.sync.dma_start(rcj32[:], rcj_ext[:])
            for hh in range(2):
                st = {"o": stp.tile([128, NT], F32, tag=f"o{hh}", name=f"o{hh}")}
                st["taun"] = stp.tile([128, NT], F16, tag=f"taun{hh}",
                                      name=f"taun{hh}")
                cds_all[hh] = stp.tile([128, NT, NC], F16, tag=f"cds{hh}",
                                       name=f"cds{hh}")
                rC_all[hh] = stp.tile([128, NT, NC], F16, tag=f"rC{hh}",
                                      name=f"rC{hh}")
                sC_all[hh] = stp.tile([128, NT, NC], F16, tag=f"sC{hh}",
                                      name=f"sC{hh}")
                c32_all[hh] = [
                    stp.tile([128, NT, NC], F32, tag=f"c32a{hh}", name=f"c32a{hh}"),
                    stp.tile([128, NT, NC], F32, tag=f"c32b{hh}", name=f"c32b{hh}"),
                ]
                st_all[hh] = st
            trs_all = [[stp.tile([PT, 128], F16, tag=f"trs{hh}_{p}",
                                 name=f"trs{hh}_{p}") for p in range(NP)]
                       for hh in range(2)]

            # ---------------- pipelined panels ----------------
            if True:
                def emit_c_quad(t, half):
                    """sim for query tile t, key half: 4 matmuls sharing the
                    stationary qT2[:, t] (via zero-padded per-head rhs),
                    then MAX8 top-8 per head straight from PSUM."""
                    pss = [psS.tile([128, 1024], F32, tag="sim", name=f"sim{_h}")
                           for _h in range(2)]
                    for qb in range(2):
                        for hh in range(2):
                            nc.tensor.matmul(
                                pss[hh][:, qb * 512:(qb + 1) * 512],
                                qT2[:, t * 128:(t + 1) * 128],
                                kZh[hh][:, half * 1024 + qb * 512:
                                        half * 1024 + (qb + 1) * 512],
                                start=True, stop=True,
                            )
                    for hh in range(2):
                        nc.vector.max(
                            cds_all[hh][:, t, half * 8:(half + 1) * 8],
                            pss[hh][:])

                def emit_chain(p, hh):
                    """tau for panel p, head hh: bitonic merge of the two
                    sorted top-8 runs, cumsum, tau = max_j (cssv_j - 1)/j."""
                    ts = slice(p * PT, (p + 1) * PT)
                    st = st_all[hh]
                    cds = cds_all[hh][:, ts, :]
                    sA = rC_all[hh][:, ts, :]
                    sB = sC_all[hh][:, ts, :]
                    ca = c32_all[hh][0][:, ts, :]
                    cb = c32_all[hh][1][:, ts, :]
                    nc.vector.tensor_copy(sA[:, :, 0:8], cds[:, :, 0:8])
                    nc.vector.tensor_copy(sA[:, :, 8:16], cds[:, :, 15:7:-1])
                    nc.vector.tensor_tensor(
                        sB[:, :, 0:8], sA[:, :, 0:8], sA[:, :, 8:16], A.max)
                    nc.vector.tensor_tensor(
                        sB[:, :, 8:16], sA[:, :, 0:8], sA[:, :, 8:16], A.min)
                    vB = sB.rearrange("p t (g w) -> p t g w", w=8)
                    vA = sA.rearrange("p t (g w) -> p t g w", w=8)
                    nc.vector.tensor_tensor(
                        vA[:, :, :, 0:4], vB[:, :, :, 0:4], vB[:, :, :, 4:8], A.max)
                    nc.vector.tensor_tensor(
                        vA[:, :, :, 4:8], vB[:, :, :, 0:4], vB[:, :, :, 4:8], A.min)
                    vA4 = sA.rearrange("p t (g w) -> p t g w", w=4)
                    vB4 = sB.rearrange("p t (g w) -> p t g w", w=4)
                    nc.vector.tensor_tensor(
                        vB4[:, :, :, 0:2], vA4[:, :, :, 0:2], vA4[:, :, :, 2:4], A.max)
                    nc.vector.tensor_tensor(
                        vB4[:, :, :, 2:4], vA4[:, :, :, 0:2], vA4[:, :, :, 2:4], A.min)
                    vB2 = sB.rearrange("p t (g w) -> p t g w", w=2)
                    vA2 = sA.rearrange("p t (g w) -> p t g w", w=2)
                    nc.vector.tensor_tensor(
                        vA2[:, :, :, 0:1], vB2[:, :, :, 0:1], vB2[:, :, :, 1:2], A.max)
                    nc.vector.tensor_tensor(
                        vA2[:, :, :, 1:2], vB2[:, :, :, 0:1], vB2[:, :, :, 1:2], A.min)
                    nc.vector.tensor_copy(ca[:, :, 0:1], sA[:, :, 0:1])
                    nc.vector.tensor_tensor(
                        ca[:, :, 1:16], sA[:, :, 1:16], sA[:, :, 0:15], A.add)
                    nc.vector.tensor_copy(cb[:, :, 0:2], ca[:, :, 0:2])
                    nc.vector.tensor_tensor(
                        cb[:, :, 2:16], ca[:, :, 2:16], ca[:, :, 0:14], A.add)
                    nc.vector.tensor_copy(ca[:, :, 0:4], cb[:, :, 0:4])
                    nc.vector.tensor_tensor(
                        ca[:, :, 4:16], cb[:, :, 4:16], cb[:, :, 0:12], A.add)
                    nc.vector.tensor_copy(cb[:, :, 0:8], ca[:, :, 0:8])
                    nc.vector.tensor_tensor(
                        cb[:, :, 8:16], ca[:, :, 8:16], ca[:, :, 0:8], A.add)
                    nc.vector.tensor_scalar(cb[:], cb[:], -1.0, None, A.add)
                    rcb = rcj32[:].unsqueeze(1).broadcast_to((128, PT, NC))
                    nc.vector.tensor_tensor(cb[:], cb[:], rcb, A.mult)
                    nc.vector.tensor_reduce(st["o"][:, ts], cb[:], AX.X, A.max)
                    nc.vector.tensor_scalar(
                        st["taun"][:, ts], st["o"][:, ts], -1.0, None, A.mult)

                def emit_tau_row(p, hh):
                    st = st_all[hh]
                    ts = slice(p * PT, (p + 1) * PT)
                    trp = psT.tile([PT, 128], F16, tag="ps", name="trp")
                    nc.tensor.transpose(trp[:], st["taun"][:, ts], idn16[:])
                    nc.scalar.activation(trs_all[hh][p][:], trp[:], AF.Copy)
                    nc.gpsimd.dma_start(
                        qAh[hh][64:65, p * 512:(p + 1) * 512], trs_all[hh][p][:])

                def emit_v_transpose(kb):
                    pt = psT.tile([128, 128], F16, tag="ps", name="ptv")
                    nc.tensor.transpose(
                        pt[:], vT[:, kb * 128:(kb + 1) * 128], idn16[:])
                    if kb % 2 == 0:
                        nc.scalar.activation(v16[:, kb, :], pt[:], AF.Copy)
                    else:
                        nc.vector.tensor_copy(v16[:, kb, :], pt[:])

                def emit_p2(p, extras):
                    """P2 for panel p; extras (C quads of p+1 etc.) spread in.
                    attnT relu-copies split scalar/DVE by panel: DVE is
                    MAX8-bound early and idle at the tail."""
                    dve_mod = (999, 8, 4, 2)[p]
                    qs = slice(p * 512, (p + 1) * 512)
                    ats = []
                    pav = psA.tile([128, 512], F32, tag="av", name="av")
                    ex = list(extras)
                    for kb in range(NKB):
                        for hh in range(2):
                            ps = psT.tile([128, 512], F32, tag="ps",
                                          name="simT")
                            nc.tensor.matmul(
                                ps[:],
                                kAh[hh][:, kb * 128:(kb + 1) * 128],
                                qAh[hh][:, qs],
                                start=True, stop=True,
                            )
                            at = atp.tile([128, 512], F16, tag="at", name="at")
                            if (kb * 2 + hh) % dve_mod == dve_mod - 1:
                                nc.vector.tensor_scalar(
                                    at[:], ps[:], 0.0, None, A.max)
                            else:
                                nc.scalar.activation(at[:], ps[:], AF.Relu)
                            ats.append((hh, at))
                        for hh, at in ats[-2:]:
                            nc.tensor.matmul(
                                pav[hh * 64:(hh + 1) * 64, :],
                                v16[:, kb, hh * 64:(hh + 1) * 64],
                                at[:],
                                start=(kb == 0), stop=(kb == NKB - 1),
                                skip_group_check=True,
                            )
                        if ex and kb in (2, 3, 5, 6, 8, 9, 11, 12):
                            ex.pop(0)()
                    nc.vector.tensor_copy(aoT[:, qs], pav[:])
                    # fused output projection for this panel
                    for rb in range(p * 4, (p + 1) * 4):
                        pso = psT.tile([128, 512], F32, tag="ps", name="pso")
                        nc.tensor.matmul(
                            pso[:], aoT[:, rb * 128:(rb + 1) * 128], wo16[:],
                            start=True, stop=True,
                        )
                        ob = op_.tile([128, 512], F16, tag="ob")
                        nc.scalar.activation(ob[:], pso[:], AF.Copy)
                        eng = nc.gpsimd if rb % 2 == 0 else nc.sync
                        eng.dma_start(out_ext[rb * 128:(rb + 1) * 128, :], ob[:])
                        if ex:
                            ex.pop(0)()
                    while ex:
                        ex.pop(0)()

                def c_quads(p):
                    return [
                        (lambda t=t, half=half: emit_c_quad(t, half))
                        for t in range(p * PT, (p + 1) * PT)
                        for half in range(2)
                    ]

                def chains(p):
                    for hh in range(2):
                        emit_chain(p, hh)
                        emit_tau_row(p, hh)

                vts = [lambda kb=kb: emit_v_transpose(kb) for kb in range(NKB)]
                q0 = c_quads(0)
                fillers = [lambda nb=nb: emit_wv_nb(nb) for nb in range(4)]
                for i, q in enumerate(q0):
                    q()
                    if i < len(fillers):
                        fillers[i]()
                psB_ctx.__exit__(None, None, None)
                psT_ctx = tc.tile_pool(name="psT", bufs=3,
                                       space=bass.MemorySpace.PSUM)
                psT = psT_ctx.__enter__()
                psA_ctx = tc.tile_pool(name="psAv", bufs=1,
                                       space=bass.MemorySpace.PSUM)
                psA = psA_ctx.__enter__()
                chains(0)
                for vt in vts:
                    vt()
                emit_p2(0, c_quads(1))
                chains(1)
                emit_p2(1, c_quads(2))
                chains(2)
                emit_p2(2, c_quads(3))
                chains(3)
                emit_p2(3, [])
                psA_ctx.__exit__(None, None, None)
                psT_ctx.__exit__(None, None, None)
                psS_ctx.__exit__(None, None, None)
            stage_ctx.__exit__(None, None, None)

    nc.compile()
    return nc


_NC_CACHE = None


def _get_nc():
    global _NC_CACHE
    if _NC_CACHE is None:
        _NC_CACHE = build()
    return _NC_CACHE


def make_in_maps(x, W_qkv, W_out, b_out):
    wq = (W_qkv[:, :512] * SCALE).astype(np.float16)
    wk = W_qkv[:, 512:1024].astype(np.float16)
    wv = W_qkv[:, 1024:1536].astype(np.float16)
    wo = W_out.astype(np.float16)
    idn = np.eye(128, dtype=np.float16)
    rcj = np.tile((1.0 / np.arange(1, NC + 1, dtype=np.float32))[None, :], (128, 1))
    xTs = [np.ascontiguousarray(x[b].T.astype(np.float16)) for b in range(B)]
    in_maps = []
    for c in range(8):
        b, hp = c // 4, c % 4
        h0 = 2 * hp
        sl = slice(h0 * 64, (h0 + 2) * 64)
        def repack(w):
            # [512, 128] head-slice -> [128 partitions, KC*128] so the SBUF
            # tile [128, KC, 128] loads as one contiguous region
            return np.ascontiguousarray(
                w[:, sl].reshape(KC, 128, 128).transpose(1, 0, 2).reshape(128, KC * 128))
        in_maps.append({
            "xT": xTs[b],
            "wq": repack(wq),
            "wk": repack(wk),
            "wv": repack(wv),
            "wo": np.ascontiguousarray(wo[sl, :]),
            "idn": idn,
            "rcj": rcj,
        })
    return in_maps


def kernel(x, W_qkv, W_out, b_out, _trace=False, _results_box=None):
    nc = _get_nc()
    in_maps = make_in_maps(x, W_qkv, W_out, b_out)
    res = run_bass_kernel_spmd(nc, in_maps, list(range(8)), trace=_trace)
    if _results_box is not None:
        _results_box.append(res)
    out = np.zeros((B, N, D), np.float32)
    for c in range(8):
        b = c // 4
        out[b] += res.results[c]["out"].astype(np.float32)
    out += b_out[None, None, :].astype(np.float32)
    return out


# revision 16
# speedup vs baseline: 1.0056x; 1.0056x over previous
"""Sparse attention (sparsemax) TRN2 kernel — 8 NeuronCores, SPMD.

Panel-pipelined v3. Core c handles batch b=c//4 and head pair
(2*(c%4), 2*(c%4)+1). Host sums 4 partials per batch + bias.

Key PE trick: matmul streaming costs ~N cycles, but a CHANGED stationary
operand serializes ~LDWEIGHTS in front (measured 485ns vs 216ns for
N=512 f16). So everything is structured to reuse the stationary operand:
  - phase C sim uses lhsT = qT2[:, t] (both heads stacked, K=128) reused
    across 8 matmuls; the per-head split moves to the rhs via
    zero-padded kZ0/kZ1 (head h's K rows live at partitions h*64..h*64+63,
    the other 64 rows are zero, so the full-K contraction yields that
    head's sim exactly).
  - projections run kc-major with 4 concurrent PSUM accumulators so each
    weight chunk is loaded once and streamed over 4 x-column blocks.
simT keeps the augmented-row trick (row 64: ones/-tau -> PSUM holds
sim - tau) but zero-pads K to 128 to stay in the fast weight-load path.

Pipeline: queries in 4 panels of 512. C(p) sim+MAX8 (top-8 per
1024-half direct from PSUM), DVE chain -> tau, then P2(p): simT,
relu-copies to attnT (split scalar/DVE), attn@v accumulation, fused
output projection. C(p+1) quads interleave into P2(p)'s PE stream.
"""
import sys

sys.path.insert(0, "/opt/trn_rl_repo")

import numpy as np
import concourse.bass as bass
import concourse.bacc as bacc
import concourse.mybir as mybir
import concourse.tile as tile
from concourse.bass_utils import run_bass_kernel_spmd

F32 = mybir.dt.float32
F16 = mybir.dt.float16
A = mybir.AluOpType
AF = mybir.ActivationFunctionType
AX = mybir.AxisListType

B, N, D = 2, 2048, 512
H, DH = 8, 64
SCALE = DH ** -0.5
NT = N // 128        # 16 query tiles
NKB = N // 128       # 16 key blocks
KC = D // 128        # 4 contraction chunks over model dim
NC = 16              # candidates per row
NP = 4               # query panels
PT = NT // NP        # 4 query tiles per panel


def build():
    nc = bacc.Bacc(None, target_bir_lowering=False)

    xT_ext = nc.declare_dram_parameter("xT", [D, N], F16, isOutput=False)
    wq_ext = nc.declare_dram_parameter("wq", [128, KC * 128], F16, isOutput=False)
    wk_ext = nc.declare_dram_parameter("wk", [128, KC * 128], F16, isOutput=False)
    wv_ext = nc.declare_dram_parameter("wv", [128, KC * 128], F16, isOutput=False)
    wo_ext = nc.declare_dram_parameter("wo", [128, D], F16, isOutput=False)
    idn_ext = nc.declare_dram_parameter("idn", [128, 128], F16, isOutput=False)
    rcj_ext = nc.declare_dram_parameter("rcj", [128, NC], F32, isOutput=False)
    out_ext = nc.declare_dram_parameter("out", [N, D], F16, isOutput=True)

    with tile.TileContext(nc) as tc:
        with (
            tc.tile_pool(name="persist", bufs=1) as pp,
            tc.tile_pool(name="statpool", bufs=1) as stp,
            tc.tile_pool(name="attnT", bufs=36) as atp,
            tc.tile_pool(name="outp", bufs=2) as op_,
        ):
            # ---------------- Phase A: loads ----------------
            qT2 = pp.tile([128, N], F16, tag="qT2")
            # per-head zero-padded K tiles for phase C (shared-lhsT trick)
            kZh = [pp.tile([128, N], F16, tag=f"kZ{hh}", name=f"kZ{hh}")
                   for hh in range(2)]
            # K=128-padded augmented tiles for simT
            kAh = [pp.tile([128, N], F16, tag=f"kA{hh}", name=f"kA{hh}")
                   for hh in range(2)]
            qAh = [pp.tile([128, N], F16, tag=f"qA{hh}", name=f"qA{hh}")
                   for hh in range(2)]
            v16 = pp.tile([128, NKB, 128], F16, tag="v16")
            aoT = pp.tile([128, N], F16, tag="aoT")
            wo16 = pp.tile([128, 512], F16, tag="wo")
            idn16 = pp.tile([128, 128], F16, tag="idn")

            stage_ctx = tc.tile_pool(name="stage", bufs=1)
            stg = stage_ctx.__enter__()
            w16 = {nm: stg.tile([128, KC, 128], F16, tag=nm, name=nm)
                   for nm in ("wk", "wq", "wv")}
            xT16 = [stg.tile([128, N], F16, tag=f"xT{kc}", name=f"xT{kc}")
                    for kc in range(KC)]
            # weights are host-repacked so each w16 tile is one contiguous
            # [128, KC*128] region; split by partition halves across the two
            # HWDGE queues. wk + xT0 gate the first projection group; xT1/xT3
            # ride the gpsimd software-DGE rings in parallel.
            nc.sync.dma_start(w16["wk"][0:64, :, :], wk_ext[0:64, :])
            nc.scalar.dma_start(w16["wk"][64:128, :, :], wk_ext[64:128, :])
            for kc in (0, 2):
                x0 = kc * 128
                nc.scalar.dma_start(xT16[kc][0:64, :],
                                    xT_ext[x0:x0 + 64, :])
                nc.sync.dma_start(xT16[kc][64:128, :],
                                  xT_ext[x0 + 64:x0 + 128, :])
                if kc == 0:
                    nc.sync.dma_start(w16["wq"][0:64, :, :], wq_ext[0:64, :])
                    nc.scalar.dma_start(w16["wq"][64:128, :, :],
                                        wq_ext[64:128, :])
            for kc in (1, 3):
                x0 = kc * 128
                nc.gpsimd.dma_start(xT16[kc][0:64, :], xT_ext[x0:x0 + 64, :])
                nc.gpsimd.dma_start(xT16[kc][64:128, :],
                                    xT_ext[x0 + 64:x0 + 128, :])
            nc.gpsimd.dma_start(w16["wv"][:, :, :], wv_ext[:, :])
            vT = stg.tile([128, N], F16, tag="vT", name="vT")
            warm = stg.tile([128, 128], F16, tag="warm", name="warm")
            nc.gpsimd.memset(warm[:], 0.25)
            nc.sync.dma_start(wo16[:], wo_ext[:])
            nc.sync.dma_start(idn16[:], idn_ext[:])
            for hh in range(2):
                nc.vector.memset(kZh[hh][(1 - hh) * 64:(2 - hh) * 64, :], 0.0)
                nc.gpsimd.memset(kAh[hh][64:128, :], 0.0)
                nc.gpsimd.memset(kAh[hh][64:65, :], 1.0)
                nc.gpsimd.memset(qAh[hh][64:128, :], 0.0)

            # ---------------- Phase B: projections ----------------
            # kc-major with 4 live accumulators: each weight chunk is the
            # stationary operand for 4 consecutive matmuls (one LDW amortized).
            # psS opens first so the PSUM stack can reuse psB's banks for
            # psT/psA after the wv group (emitted inside the C(0) lead-in).
            psS_ctx = tc.tile_pool(name="psS", bufs=2, space=bass.MemorySpace.PSUM)
            psS = psS_ctx.__enter__()
            psB_ctx = tc.tile_pool(name="psB", bufs=4, space=bass.MemorySpace.PSUM)
            psB = psB_ctx.__enter__()
            if True:
                # HAM warm-up: keep the PE busy >3.4us during the load wait so
                # the clock gate opens before the projection matmuls arrive
                wps = psB.tile([128, 128], F32, tag="pskq", name="wps")
                for wi in range(28):
                    nc.tensor.matmul(wps[:], warm[:], warm[:],
                                     start=(wi == 0), stop=(wi == 27),
                                     skip_group_check=True)
                wsink = stg.tile([128, 8], F16, tag="wsink", name="wsink")
                nc.scalar.activation(wsink[:], wps[:, 0:8], AF.Copy)
                for nm in ("wk", "wq"):
                    ps4 = [psB.tile([128, 512], F32, tag="pskq", name=f"ps{nb}")
                           for nb in range(4)]
                    for kc in range(KC):
                        for nb in range(4):
                            nc.tensor.matmul(
                                ps4[nb][:], w16[nm][:, kc, :],
                                xT16[kc][:, nb * 512:(nb + 1) * 512],
                                start=(kc == 0), stop=(kc == KC - 1),
                                skip_group_check=True,
                            )
                    for nb in range(4):
                        cols = slice(nb * 512, (nb + 1) * 512)
                        if nm == "wk":
                            # split to the two zero-padded per-head tiles
                            nc.scalar.activation(
                                kZh[0][0:64, cols], ps4[nb][0:64, :], AF.Copy)
                            nc.vector.tensor_copy(
                                kZh[1][64:128, cols], ps4[nb][64:128, :])
                        else:
                            if nb % 2 == 0:
                                nc.scalar.activation(
                                    qT2[:, cols], ps4[nb][:], AF.Copy)
                            else:
                                nc.vector.tensor_copy(qT2[:, cols], ps4[nb][:])
                for hh in range(2):
                    nc.sync.dma_start(kAh[hh][0:64, :],
                                      kZh[hh][hh * 64:(hh + 1) * 64, :])
                    nc.gpsimd.dma_start(qAh[hh][0:64, :],
                                        qT2[hh * 64:(hh + 1) * 64, :])

                def emit_wv_nb(nb):
                    ps = psB.tile([128, 512], F32, tag="pskq", name="psv")
                    for kc in range(KC):
                        nc.tensor.matmul(
                            ps[:], w16["wv"][:, kc, :],
                            xT16[kc][:, nb * 512:(nb + 1) * 512],
                            start=(kc == 0), stop=(kc == KC - 1),
                        )
                    nc.scalar.activation(
                        vT[:, nb * 512:(nb + 1) * 512], ps[:], AF.Copy)

            # ---------------- tau state ----------------
            st_all, cds_all, rC_all, sC_all, c32_all = {}, {}, {}, {}, {}
            rcj32 = stp.tile([128, NC], F32, tag="rcj", name="rcj")
            nc.sync.dma_start(rcj32[:], rcj_ext[:])
            for hh in range(2):
                st = {"o": stp.tile([128, NT], F32, tag=f"o{hh}", name=f"o{hh}")}
                st["taun"] = stp.tile([128, NT], F16, tag=f"taun{hh}",
                                      name=f"taun{hh}")
                cds_all[hh] = stp.tile([128, NT, NC], F16, tag=f"cds{hh}",
                                       name=f"cds{hh}")
                rC_all[hh] = stp.tile([128, NT, NC], F16, tag=f"rC{hh}",
                                      name=f"rC{hh}")
                sC_all[hh] = stp.tile([128, NT, NC], F16, tag=f"sC{hh}",
                                      name=f"sC{hh}")
                c32_all[hh] = [
                    stp.tile([128, NT, NC], F32, tag=f"c32a{hh}", name=f"c32a{hh}"),
                    stp.tile([128, NT, NC], F32, tag=f"c32b{hh}", name=f"c32b{hh}"),
                ]
                st_all[hh] = st
            trs_all = [[stp.tile([PT, 128], F16, tag=f"trs{hh}_{p}",
                                 name=f"trs{hh}_{p}") for p in range(NP)]
                       for hh in range(2)]

            # ---------------- pipelined panels ----------------
            if True:
                def emit_c_quad(t, half):
                    """sim for query tile t, key half: 4 matmuls sharing the
                    stationary qT2[:, t] (via zero-padded per-head rhs),
                    then MAX8 top-8 per head straight from PSUM."""
                    pss = [psS.tile([128, 1024], F32, tag="sim", name=f"sim{_h}")
                           for _h in range(2)]
                    for qb in range(2):
                        for hh in range(2):
                            nc.tensor.matmul(
                                pss[hh][:, qb * 512:(qb + 1) * 512],
                                qT2[:, t * 128:(t + 1) * 128],
                                kZh[hh][:, half * 1024 + qb * 512:
                                        half * 1024 + (qb + 1) * 512],
                                start=True, stop=True,
                            )
                    for hh in range(2):
                        nc.vector.max(
                            cds_all[hh][:, t, half * 8:(half + 1) * 8],
                            pss[hh][:])

                def emit_chain(p, hh):
                    """tau for panel p, head hh: bitonic merge of the two
                    sorted top-8 runs, cumsum, tau = max_j (cssv_j - 1)/j."""
                    ts = slice(p * PT, (p + 1) * PT)
                    st = st_all[hh]
                    cds = cds_all[hh][:, ts, :]
                    sA = rC_all[hh][:, ts, :]
                    sB = sC_all[hh][:, ts, :]
                    ca = c32_all[hh][0][:, ts, :]
                    cb = c32_all[hh][1][:, ts, :]
                    nc.vector.tensor_copy(sA[:, :, 0:8], cds[:, :, 0:8])
                    nc.vector.tensor_copy(sA[:, :, 8:16], cds[:, :, 15:7:-1])
                    nc.vector.tensor_tensor(
                        sB[:, :, 0:8], sA[:, :, 0:8], sA[:, :, 8:16], A.max)
                    nc.vector.tensor_tensor(
                        sB[:, :, 8:16], sA[:, :, 0:8], sA[:, :, 8:16], A.min)
                    vB = sB.rearrange("p t (g w) -> p t g w", w=8)
                    vA = sA.rearrange("p t (g w) -> p t g w", w=8)
                    nc.vector.tensor_tensor(
                        vA[:, :, :, 0:4], vB[:, :, :, 0:4], vB[:, :, :, 4:8], A.max)
                    nc.vector.tensor_tensor(
                        vA[:, :, :, 4:8], vB[:, :, :, 0:4], vB[:, :, :, 4:8], A.min)
                    vA4 = sA.rearrange("p t (g w) -> p t g w", w=4)
                    vB4 = sB.rearrange("p t (g w) -> p t g w", w=4)
                    nc.vector.tensor_tensor(
                        vB4[:, :, :, 0:2], vA4[:, :, :, 0:2], vA4[:, :, :, 2:4], A.max)
                    nc.vector.tensor_tensor(
                        vB4[:, :, :, 2:4], vA4[:, :, :, 0:2], vA4[:, :, :, 2:4], A.min)
                    vB2 = sB.rearrange("p t (g w) -> p t g w", w=2)
                    vA2 = sA.rearrange("p t (g w) -> p t g w", w=2)
                    nc.vector.tensor_tensor(
                        vA2[:, :, :, 0:1], vB2[:, :, :, 0:1], vB2[:, :, :, 1:2], A.max)
                    nc.vector.tensor_tensor(
                        vA2[:, :, :, 1:2], vB2[:, :, :, 0:1], vB2[:, :, :, 1:2], A.min)
                    nc.vector.tensor_copy(ca[:, :, 0:1], sA[:, :, 0:1])
                    nc.vector.tensor_tensor(
                        ca[:, :, 1:16], sA[:, :, 1:16], sA[:, :, 0:15], A.add)
                    nc.vector.tensor_copy(cb[:, :, 0:2], ca[:, :, 0:2])
                    nc.vector.tensor_tensor(
                        cb[:, :, 2:16], ca[:, :, 2:16], ca[:, :, 0:14], A.add)
                    nc.vector.tensor_copy(ca[:, :, 0:4], cb[:, :, 0:4])
                    nc.vector.tensor_tensor(
                        ca[:, :, 4:16], cb[:, :, 4:16], cb[:, :, 0:12], A.add)
                    nc.vector.tensor_copy(cb[:, :, 0:8], ca[:, :, 0:8])
                    nc.vector.tensor_tensor(
                        cb[:, :, 8:16], ca[:, :, 8:16], ca[:, :, 0:8], A.add)
                    nc.vector.tensor_scalar(cb[:], cb[:], -1.0, None, A.add)
                    rcb = rcj32[:].unsqueeze(1).broadcast_to((128, PT, NC))
                    nc.vector.tensor_tensor(cb[:], cb[:], rcb, A.mult)
                    nc.vector.tensor_reduce(st["o"][:, ts], cb[:], AX.X, A.max)
                    nc.vector.tensor_scalar(
                        st["taun"][:, ts], st["o"][:, ts], -1.0, None, A.mult)

                def emit_tau_row(p, hh):
                    st = st_all[hh]
                    ts = slice(p * PT, (p + 1) * PT)
                    trp = psT.tile([PT, 128], F16, tag="ps", name="trp")
                    nc.tensor.transpose(trp[:], st["taun"][:, ts], idn16[:])
                    nc.scalar.activation(trs_all[hh][p][:], trp[:], AF.Copy)
                    nc.gpsimd.dma_start(
                        qAh[hh][64:65, p * 512:(p + 1) * 512], trs_all[hh][p][:])

                def emit_v_transpose(kb):
                    pt = psT.tile([128, 128], F16, tag="ps", name="ptv")
                    nc.tensor.transpose(
                        pt[:], vT[:, kb * 128:(kb + 1) * 128], idn16[:])
                    if kb % 2 == 0:
                        nc.scalar.activation(v16[:, kb, :], pt[:], AF.Copy)
                    else:
                        nc.vector.tensor_copy(v16[:, kb, :], pt[:])

                def emit_p2(p, extras):
                    """P2 for panel p; extras (C quads of p+1 etc.) spread in.
                    attnT relu-copies split scalar/DVE by panel: DVE is
                    MAX8-bound early and idle at the tail."""
                    dve_mod = (999, 8, 4, 2)[p]
                    qs = slice(p * 512, (p + 1) * 512)
                    ats = []
                    pav = psA.tile([128, 512], F32, tag="av", name="av")
                    ex = list(extras)
                    for kb in range(NKB):
                        for hh in range(2):
                            ps = psT.tile([128, 512], F32, tag="ps",
                                          name="simT")
                            nc.tensor.matmul(
                                ps[:],
                                kAh[hh][:, kb * 128:(kb + 1) * 128],
                                qAh[hh][:, qs],
                                start=True, stop=True,
                            )
                            at = atp.tile([128, 512], F16, tag="at", name="at")
                            if (kb * 2 + hh) % dve_mod == dve_mod - 1:
                                nc.vector.tensor_scalar(
                                    at[:], ps[:], 0.0, None, A.max)
                            else:
                                nc.scalar.activation(at[:], ps[:], AF.Relu)
                            ats.append((hh, at))
                        for hh, at in ats[-2:]:
                            nc.tensor.matmul(
                                pav[hh * 64:(hh + 1) * 64, :],
                                v16[:, kb, hh * 64:(hh + 1) * 64],
                                at[:],
                                start=(kb == 0), stop=(kb == NKB - 1),
                                skip_group_check=True,
                            )
                        if ex and kb in (2, 3, 5, 6, 8, 9, 11, 12):
                            ex.pop(0)()
                    nc.vector.tensor_copy(aoT[:, qs], pav[:])
                    # fused output projection for this panel
                    for rb in range(p * 4, (p + 1) * 4):
                        pso = psT.tile([128, 512], F32, tag="ps", name="pso")
                        nc.tensor.matmul(
                            pso[:], aoT[:, rb * 128:(rb + 1) * 128], wo16[:],
                            start=True, stop=True,
                        )
                        ob = op_.tile([128, 512], F16, tag="ob")
                        nc.scalar.activation(ob[:], pso[:], AF.Copy)
                        eng = nc.gpsimd if rb % 2 == 0 else nc.sync
                        eng.dma_start(out_ext[rb * 128:(rb + 1) * 128, :], ob[:])
                        if ex:
                            ex.pop(0)()
                    while ex:
                        ex.pop(0)()

                def c_quads(p):
                    return [
                        (lambda t=t, half=half: emit_c_quad(t, half))
                        for t in range(p * PT, (p + 1) * PT)
                        for half in range(2)
                    ]

                def chains(p):
                    for hh in range(2):
                        emit_chain(p, hh)
                        emit_tau_row(p, hh)

                vts = [lambda kb=kb: emit_v_transpose(kb) for kb in range(NKB)]
                q0 = c_quads(0)
                fillers = [lambda nb=nb: emit_wv_nb(nb) for nb in range(4)]
                for i, q in enumerate(q0):
                    q()
                    if i < len(fillers):
                        fillers[i]()
                psB_ctx.__exit__(None, None, None)
                psT_ctx = tc.tile_pool(name="psT", bufs=3,
                                       space=bass.MemorySpace.PSUM)
                psT = psT_ctx.__enter__()
                psA_ctx = tc.tile_pool(name="psAv", bufs=1,
                                       space=bass.MemorySpace.PSUM)
                psA = psA_ctx.__enter__()
                chains(0)
                for vt in vts:
                    vt()
                emit_p2(0, c_quads(1))
                chains(1)
                emit_p2(1, c_quads(2))
                chains(2)
                emit_p2(2, c_quads(3))
                chains(3)
                emit_p2(3, [])
                psA_ctx.__exit__(None, None, None)
                psT_ctx.__exit__(None, None, None)
                psS_ctx.__exit__(None, None, None)
            stage_ctx.__exit__(None, None, None)

    nc.compile()
    return nc


_NC_CACHE = None


def _get_nc():
    global _NC_CACHE
    if _NC_CACHE is None:
        _NC_CACHE = build()
    return _NC_CACHE


def make_in_maps(x, W_qkv, W_out, b_out):
    wq = (W_qkv[:, :512] * SCALE).astype(np.float16)
    wk = W_qkv[:, 512:1024].astype(np.float16)
    wv = W_qkv[:, 1024:1536].astype(np.float16)
    wo = W_out.astype(np.float16)
    idn = np.eye(128, dtype=np.float16)
    rcj = np.tile((1.0 / np.arange(1, NC + 1, dtype=np.float32))[None, :], (128, 1))
    xTs = [np.ascontiguousarray(x[b].T.astype(np.float16)) for b in range(B)]
    in_maps = []
    for c in range(8):
        b, hp = c // 4, c % 4
        h0 = 2 * hp
        sl = slice(h0 * 64, (h0 + 2) * 64)
        def repack(w):
            # [512, 128] head-slice -> [128 partitions, KC*128] so the SBUF
            # tile [128, KC, 128] loads as one contiguous region
            return np.ascontiguousarray(
                w[:, sl].reshape(KC, 128, 128).transpose(1, 0, 2).reshape(128, KC * 128))
        in_maps.append({
            "xT": xTs[b],
            "wq": repack(wq),
            "wk": repack(wk),
            "wv": repack(wv),
            "wo": np.ascontiguousarray(wo[sl, :]),
            "idn": idn,
            "rcj": rcj,
        })
    return in_maps


def kernel(x, W_qkv, W_out, b_out, _trace=False, _results_box=None):
    nc = _get_nc()
    in_maps = make_in_maps(x, W_qkv, W_out, b_out)
    res = run_bass_kernel_spmd(nc, in_maps, list(range(8)), trace=_trace)
    if _results_box is not None:
        _results_box.append(res)
    out = np.zeros((B, N, D), np.float32)
    for c in range(8):
        b = c // 4
        out[b] += res.results[c]["out"].astype(np.float32)
    out += b_out[None, None, :].astype(np.float32)
    return out
